# revision 34
# baseline (speedup 1.0000x reference)
"""Trainium2 Bass kernel for nn_Attention_83004537963197.

LayerNorm -> QKV projection -> 8-head attention (head_dim=16) -> output
projection, x[16, 1024, 1024] f32.  Data-parallel over batch: 2 batches
per NeuronCore across 8 cores, no collectives.

v4 structure (evidence-driven, see trace analysis):
  - Software-pipelined attention chunks: scores(k+1) are EMITTED before
    attnv(k), so the in-order PE queue computes the next chunk's scores
    while exp(k) runs and the exp stream never waits on the PE
    (the naive order scores(k)->attnv(k)->scores(k+1) serializes
    attnv(k) [waits exp(k)] ahead of scores(k+1), defeating the
    double-buffered score banks).
  - One activation table set for the entire kernel: LN rstd is computed
    as exp(-0.5*ln(var+eps)) and the act-table list is filtered so Ln
    and Exp both resolve to natural_log_exp_and_others (v1 thrashed
    exp<->sqrt ACT_TABLE_LOADs mid-stream).
  - x^T via PE matmuls against identity (DMA-transpose serializes
    against all concurrent DMA traffic - measured 8us per transpose -
    so it's unusable mid-stream).
  - Attention chunk order interleaves jt-halves and regions:
    (r0,jt0-3),(r1,jt0-3),(r0,jt4-7),(r1,jt4-7) per i-half, so the
    first 16 chunks only need the nt=0 halves of q^T/k^T and attention
    starts as soon as x tiles 0-3 are normalized+transposed; the nt=1
    prep runs as fillers.
  - Both regions of an i-half finish together -> single-pass output
    projection (2 accumulating matmuls), serial tail is only the last
    i-half's 8 chunks.
  - exp N=1024 from double-buffered [P,2,512]f32 score tiles (PSUM
    banks 0-3; oT banks 4-5 (two concurrent groups); small pool 6-7),
    scores 2-way row-tiled (K=16 at 32c), attnv 2-way col-tiled,
    softmax row-sums via the ones-column trick (v_aug col 0 = 1),
    reciprocal + stride-0-DRAM-broadcast normalize with a single
    full-width PSUM evacuation.
  - Per-tile rstd during the batch-0 ramp (ScalarE idle), batched per-4
    for batch 1 (protects the exp stream).
  - Output stored bf16 (host upcasts).
"""

from contextlib import ExitStack

import numpy as np
import ml_dtypes

import concourse.bass as bass
import concourse.tile as tile
from concourse import bacc, mybir, hw_specs
from concourse.bass_utils import run_bass_kernel_spmd

# ---- single activation-table-set patch -------------------------------
_orig_get_act_tables = hw_specs.get_activation_tables


def _patched_get_act_tables(arch):
    tabs = _orig_get_act_tables(arch)
    EXP = mybir.ActivationFunctionType.Exp
    LN = mybir.ActivationFunctionType.Ln
    out = {}
    for name, funcs in tabs.items():
        if name != "natural_log_exp_and_others":
            funcs = funcs - {EXP, LN}
        out[name] = funcs
    return out


hw_specs.get_activation_tables = _patched_get_act_tables
import concourse.bacc as _bacc_mod  # noqa: E402

_bacc_mod.get_activation_tables = _patched_get_act_tables

F32 = mybir.dt.float32
BF16 = mybir.dt.bfloat16

B, N, EMB = 16, 1024, 1024
HEADS, INNER = 8, 128
HD = INNER // HEADS            # 16
SCALE = INNER ** -0.5
EPS = 1e-5
NCORES = 8
NB = B // NCORES               # batches per core
P = 128
NT = EMB // P                  # 8 tiles along emb / n

Sub = mybir.AluOpType.subtract
Mult = mybir.AluOpType.mult
Add = mybir.AluOpType.add
AF = mybir.ActivationFunctionType

_CACHE = {}


def _build():
    nc = bacc.Bacc(None, target_bir_lowering=False)

    xs_h = nc.declare_dram_parameter("xs", [NB, N, EMB], F32, isOutput=False)
    wqk_h = nc.declare_dram_parameter("wqk", [P, NT, 2, P], BF16, isOutput=False)
    bqk_h = nc.declare_dram_parameter("bqk", [P, 2], F32, isOutput=False)
    wv_h = nc.declare_dram_parameter("wv", [P, NT, P], BF16, isOutput=False)
    bv_h = nc.declare_dram_parameter("bv", [1, P], BF16, isOutput=False)
    wpj_h = nc.declare_dram_parameter("wproj", [P, 2, EMB], BF16, isOutput=False)
    id_h = nc.declare_dram_parameter("ident", [P, P], BF16, isOutput=False)
    out_h = nc.declare_dram_parameter("out", [NB, N, EMB], BF16, isOutput=True)

    with tile.TileContext(nc) as tc, ExitStack() as ctx:
        ent = ctx.enter_context
        const = ent(tc.tile_pool(name="const", bufs=1))
        xpool = ent(tc.tile_pool(name="xpool", bufs=8))
        xnpool = ent(tc.tile_pool(name="xnpool", bufs=3))
        stat = ent(tc.tile_pool(name="stat", bufs=4))
        xT_pool = ent(tc.tile_pool(name="xT", bufs=2))
        qk_pool = ent(tc.tile_pool(name="qk", bufs=2))
        v_pool = ent(tc.tile_pool(name="vp", bufs=2))
        e_pool = ent(tc.tile_pool(name="ep", bufs=4))
        o_pool = ent(tc.tile_pool(name="op", bufs=2))
        nrm_pool = ent(tc.tile_pool(name="nrm", bufs=2))
        fin_pool = ent(tc.tile_pool(name="fin", bufs=4))
        dram_pool = ent(tc.tile_pool(name="dsc", bufs=2, space="DRAM"))
        # PSUM: exactly 8 banks
        ps_scores = ent(tc.tile_pool(name="psc", bufs=1, space="PSUM"))  # 0-3
        ps_out = ent(tc.tile_pool(name="pso", bufs=2, space="PSUM"))     # 4-5
        ps_small = ent(tc.tile_pool(name="pss", bufs=2, space="PSUM"))   # 6-7

        # ---- constants (scalar queue; sync/gpsimd lead with x tiles) ----
        wqk_sb = const.tile([P, NT, 2, P], BF16)
        nc.scalar.dma_start(out=wqk_sb, in_=wqk_h[:])
        bqk_sb = const.tile([P, 2], F32)
        nc.scalar.dma_start(out=bqk_sb, in_=bqk_h[:])
        wv_sb = const.tile([P, NT, P], BF16)
        nc.gpsimd.dma_start(out=wv_sb, in_=wv_h[:])
        bv_sb = const.tile([1, P], BF16)
        nc.gpsimd.dma_start(out=bv_sb, in_=bv_h[:])
        wpj_sb = const.tile([P, 2, EMB], BF16)
        nc.gpsimd.dma_start(out=wpj_sb, in_=wpj_h[:])
        id_sb = const.tile([P, P], BF16)
        nc.scalar.dma_start(out=id_sb, in_=id_h[:])
        eps_sb = const.tile([P, 1], F32)
        nc.vector.memset(eps_sb, EPS)
        ones1_sb = const.tile([1, P], BF16)
        nc.vector.memset(ones1_sb, 1.0)
        U32 = mybir.dt.uint32
        magic_sb = const.tile([P, 4], U32)
        nc.vector.memset(magic_sb, 0x5F3759DF)
        c15_sb = const.tile([P, 4], F32)
        nc.vector.memset(c15_sb, 1.5)

        st8 = {0: {}, 1: {}}   # per-batch live tiles

        # ---------------- prep: LN / transpose / qkv / v ----------------

        def _state(b):
            s = st8[b]
            if s.get("xT") is None:
                s["xT"] = xT_pool.tile([P, NT, N], BF16, tag="xTt", name="xTt")
                s["xraw"] = [None] * NT
                s["mv4"] = [None, None]
                s["rstd4"] = [None, None]
            return s

        def emit_x_load(b, it, q=0, split=False):
            s = _state(b)
            xt = xpool.tile([P, EMB], F32, tag="xt", name="xt")
            if split:
                nc.sync.dma_start(out=xt[:, 0:512],
                                  in_=xs_h[b, it * P:(it + 1) * P, 0:512])
                nc.gpsimd.dma_start(out=xt[:, 512:1024],
                                    in_=xs_h[b, it * P:(it + 1) * P, 512:1024])
            else:
                eng = (nc.sync, nc.gpsimd, nc.scalar)[q]
                eng.dma_start(out=xt, in_=xs_h[b, it * P:(it + 1) * P, :])
            s["xraw"][it] = xt

        def emit_junk(tgt2d, n, nn=64):
            # HAM warm-up padding for the ramp: full-K matmuls on weight
            # data into a scratch PSUM region, so the otherwise-idle PE
            # registers activity and unthrottles before the real work.
            for _ in range(n):
                nc.tensor.matmul(
                    tgt2d[:, 0:nn], wqk_sb[:, 0, 0, :],
                    wqk_sb[:, 0, 0, 0:nn],
                    start=True, stop=True, skip_group_check=True)

        def emit_ln_stats(b, it):
            s = _state(b)
            g, k = it // 4, it % 4
            if k == 0:
                s["mv4"][g] = stat.tile([P, 4, 2], F32, tag="mv4", name="mv4")
                s["rstd4"][g] = stat.tile([P, 4], F32, tag="rstd4", name="rstd4")
            xt = s["xraw"][it]
            st = stat.tile([P, 2, 6], F32, tag="st")
            nc.vector.bn_stats(out=st[:, 0, :], in_=xt[:, 0:512])
            nc.vector.bn_stats(out=st[:, 1, :], in_=xt[:, 512:1024])
            nc.vector.bn_aggr(out=s["mv4"][g][:, k, :], in_=st)

        def emit_rstd_dve(b, g):
            # rstd = rsqrt(var+eps) entirely on the vector engine (magic
            # initial guess + 2 Newton iterations, ~5e-6 rel err): keeps
            # the scalar queue free for the exp stream.
            s = st8[b]
            Sh = mybir.AluOpType.logical_shift_right
            vpe = stat.tile([P, 4], F32, tag="vpe")
            nc.vector.tensor_scalar(out=vpe, in0=s["mv4"][g][:, :, 1],
                                    scalar1=EPS, scalar2=None, op0=Add)
            vh = stat.tile([P, 4], F32, tag="vh")
            nc.vector.tensor_scalar(out=vh, in0=s["mv4"][g][:, :, 1],
                                    scalar1=EPS, scalar2=0.5,
                                    op0=Add, op1=Mult)
            y0b = stat.tile([P, 4], U32, tag="y0b")
            nc.vector.tensor_scalar(out=y0b, in0=vpe.bitcast(U32), scalar1=1,
                                    scalar2=None, op0=Sh)
            nc.vector.tensor_tensor(out=y0b, in0=magic_sb, in1=y0b,
                                    op=Sub)
            y = y0b.bitcast(F32)
            t1 = stat.tile([P, 4], F32, tag="nt1")
            t2 = stat.tile([P, 4], F32, tag="nt2")
            for i in range(2):
                dst = s["rstd4"][g] if i == 1 else t2
                nc.vector.tensor_tensor(out=t1, in0=y, in1=y, op=Mult)
                nc.vector.tensor_tensor(out=t1, in0=t1, in1=vh, op=Mult)
                nc.vector.tensor_tensor(out=t1, in0=c15_sb, in1=t1, op=Sub)
                nc.vector.tensor_tensor(out=dst, in0=y, in1=t1, op=Mult)
                y = dst

        def emit_rstd(b, g, k=None):
            # rstd = exp(-0.5*ln(var+eps)); per-tile during the batch-0
            # ramp (ScalarE idle), batched per-4 as a batch-1 filler.
            s = st8[b]
            sl = slice(0, 4) if k is None else slice(k, k + 1)
            lnv = stat.tile([P, 4], F32, tag="lnv")
            nc.scalar.activation(out=lnv[:, sl], in_=s["mv4"][g][:, sl, 1],
                                 func=AF.Ln, bias=eps_sb)
            nc.scalar.activation(out=s["rstd4"][g][:, sl], in_=lnv[:, sl],
                                 func=AF.Exp, scale=-0.5)

        def emit_ln_norm(b, it):
            s = st8[b]
            g, k = it // 4, it % 4
            xn = xnpool.tile([P, EMB], BF16, tag="xn")
            nc.vector.tensor_scalar(
                out=xn, in0=s["xraw"][it], scalar1=s["mv4"][g][:, k, 0:1],
                scalar2=s["rstd4"][g][:, k:k + 1], op0=Sub, op1=Mult)
            s["xraw"][it] = xn     # replaced by normalized bf16

        def emit_tp(b, it):
            # transpose via PE matmul against identity; evacuation on
            # ScalarE for batch 0 (idle before the exp stream), DVE for
            # batch 1 (ScalarE is the exp stream then)
            s = st8[b]
            xT = s["xT"]
            xn = s["xraw"][it]
            for eg in range(2):
                tp = ps_small.tile([P, 4, P], F32, tag="smallps")
                for kk in range(4):
                    et = 4 * eg + kk
                    nc.tensor.matmul(
                        tp[:, kk, :], xn[:, et * P:(et + 1) * P], id_sb,
                        start=True, stop=True)
                dst = xT[:, 4 * eg:4 * eg + 4, it * P:(it + 1) * P]
                if b == 0:
                    nc.scalar.copy(out=dst, in_=tp)
                else:
                    nc.vector.tensor_copy(out=dst, in_=tp)

        def emit_qk_chunk(b, t, nt):
            # compact q^T/k^T half [128 rows = 8 heads x 16, 512 n], then
            # relocate this half's head rows into the 32-aligned region
            # layout (8 [16,512] DMAs; k on scalar / q on sync for batch 0
            # so both planes relocate in parallel; gpsimd for batch 1).
            s = st8[b]
            if s.get("qkc") is None:
                s["qkc"] = qk_pool.tile([P, 2, N], BF16, tag="qkc", name="qkc")
                s["qT"] = qk_pool.tile([P, 2, N], BF16, tag="qT", name="qT")
                s["kT"] = qk_pool.tile([P, 2, N], BF16, tag="kT", name="kT")
            xT = s["xT"]
            ps = ps_small.tile([P, 512], F32, tag="smallps")
            for et in range(NT):
                nc.tensor.matmul(
                    ps, wqk_sb[:, et, t, :],
                    xT[:, et, nt * 512:(nt + 1) * 512],
                    start=(et == 0), stop=(et == NT - 1))
            nc.vector.tensor_scalar(
                out=s["qkc"][:, t, nt * 512:(nt + 1) * 512], in0=ps,
                scalar1=bqk_sb[:, t:t + 1], scalar2=None, op0=Add)
            dst = s["qT"] if t == 0 else s["kT"]
            if b == 0:
                engs = (nc.scalar, nc.sync) if t == 1 else (nc.sync, nc.scalar)
            else:
                engs = (nc.gpsimd, nc.sync)
            for h in range(HEADS):
                r, c = h // 4, h % 4
                engs[h // 4].dma_start(
                    out=dst[32 * c:32 * c + HD, r, nt * 512:(nt + 1) * 512],
                    in_=s["qkc"][HD * h:HD * (h + 1), t,
                                 nt * 512:(nt + 1) * 512])

        def emit_v_chunk(b, jt):
            s = st8[b]
            if s.get("v") is None:
                s["v"] = v_pool.tile([P, NT, HEADS, 32], BF16, tag="vt", name="vt")
                nc.gpsimd.memset(s["v"], 0.0)
                nc.gpsimd.memset(s["v"][:, :, :, 0:1], 1.0)
            xT = s["xT"]
            ps = ps_small.tile([P, P], F32, tag="smallps")
            for et in range(NT):
                nc.tensor.matmul(
                    ps, xT[:, et, jt * P:(jt + 1) * P], wv_sb[:, et, :],
                    start=(et == 0), stop=False)
            nc.tensor.matmul(ps, ones1_sb, bv_sb, start=False, stop=True)
            nc.vector.tensor_copy(
                out=s["v"][:, jt, :, 1:17],
                in_=ps[:].rearrange("p (h d) -> p h d", d=16))

        # ---------------- projection (single pass, both regions) --------

        def emit_proj(b, it, nt, q=0):
            s = st8[b]
            ps = ps_small.tile([P, 512], F32, tag="smallps")
            for r in range(2):
                nc.tensor.matmul(
                    ps, s["o"][r][:, it * P:(it + 1) * P],
                    wpj_sb[:, r, nt * 512:(nt + 1) * 512],
                    start=(r == 0), stop=(r == 1))
            fin = fin_pool.tile([P, 512], BF16, tag="fin")
            nc.vector.tensor_copy(out=fin, in_=ps)
            eng = (nc.sync, nc.gpsimd, nc.scalar)[q]
            eng.dma_start(
                out=out_h[b, it * P:(it + 1) * P, nt * 512:(nt + 1) * 512],
                in_=fin)

        # ---------------- attention ----------------

        def emit_normalize(b, r, ih, oT_ps, last=False):
            # oT_ps [P, 512] f32: rows 32c = softmax row sums (ones-column
            # trick).  One full-tile evacuation, reciprocal on [P,16],
            # stride-0 DRAM broadcast, then one multiply.  Scratch hops on
            # sync (batch 0) / gpsimd (batch 1); the very last group uses
            # the scalar queue (exp stream is over, HWDGE is lower latency).
            s = st8[b]
            if s["o"][r] is None:
                s["o"][r] = o_pool.tile([P, N], BF16, tag="oT", name="oT")
            if last:
                eng = nc.scalar
            else:
                eng = nc.sync if b == 0 else nc.gpsimd
            i0 = ih * 512
            srow = nrm_pool.tile([P, 512], F32, tag="srow")
            nc.vector.tensor_copy(out=srow, in_=oT_ps)
            scr1 = dram_pool.tile([4, 512], F32, tag="scr1")
            eng.dma_start(out=scr1, in_=srow[0::32, :])
            cmp = nrm_pool.tile([P, 16], F32, tag="cmp")
            flat = scr1[:].rearrange("a (pp cc) -> (a pp) cc", cc=16)
            eng.dma_start(out=cmp, in_=flat)
            rec = nrm_pool.tile([P, 16], F32, tag="rec")
            nc.vector.reciprocal(out=rec, in_=cmp)
            scr2 = dram_pool.tile([4, 512], F32, tag="scr2")
            eng.dma_start(
                out=scr2[:].rearrange("a (pp cc) -> (a pp) cc", cc=16),
                in_=rec)
            rep = nrm_pool.tile([P, 512], F32, tag="rep")
            bengs = (nc.scalar, nc.sync, nc.gpsimd, nc.scalar) if last \
                else (eng, eng, eng, eng)
            for c in range(4):
                src = scr2[c:c + 1, :]
                bcast = bass.AP(
                    tensor=src.tensor, offset=src.offset,
                    ap=[[0, 32]] + list(src.ap[1:]))
                bengs[c].dma_start(out=rep[32 * c:32 * c + 32, :], in_=bcast)
            nc.vector.tensor_mul(s["o"][r][:, i0:i0 + 512], oT_ps, rep)

        def emit_attention(b, fillers, pops):
            # chunk (r, ih, jt): 4 heads per chunk.  Scores are 4-way
            # row-tiled (K=16 at 32c) into a single [P,4,512]f32 tile
            # (banks 0-3), one N=2048 exp, then 4-way col-tiled attnv
            # into oT.  Chunk order interleaves regions and jt-halves per
            # i-half so chunks 1-8 need only the nt=0 q/k halves.
            # Scores(k+1) are emitted BEFORE attnv(k) so the PE overlaps
            # the next chunk's scores with exp(k) where possible.
            s = st8[b]
            s["o"] = [None, None]
            chunks = []
            for ih in range(2):
                for half in range(2):
                    for r in range(2):
                        for jt in range(4 * half, 4 * half + 4):
                            chunks.append((r, ih, jt))
            nchunks = len(chunks)           # 32
            oT = {}
            sc_of = {}
            fi = [0]

            def fill(k):
                want = pops[k] if k < len(pops) else 1
                for _ in range(want):
                    if fi[0] < len(fillers):
                        f = fillers[fi[0]]
                        fi[0] += 1
                        if f is not None:
                            f()

            def emit_scores(idx):
                r, ih, jt = chunks[idx]
                sc = ps_scores.tile([P, 4, 512], F32, tag="sc", name="sc")
                for c in range(4):
                    nc.tensor.matmul(
                        sc[:, c, :],
                        s["kT"][32 * c:32 * c + HD, r, jt * P:(jt + 1) * P],
                        s["qT"][32 * c:32 * c + HD, r,
                                ih * 512:(ih + 1) * 512],
                        start=True, stop=True,
                        tile_position=(32 * c, 0))
                sc_of[idx] = sc

            def emit_exp_attnv(idx):
                r, ih, jt = chunks[idx]
                if (r, ih) not in oT:
                    oT[(r, ih)] = ps_out.tile([P, 512], F32, tag="oTps",
                                              name="oTps")
                sc = sc_of.pop(idx)
                E = e_pool.tile([P, 4, 512], BF16, tag="E")
                nc.scalar.activation(out=E, in_=sc, func=AF.Exp)
                for c in range(4):
                    h = 4 * r + c
                    nc.tensor.matmul(
                        oT[(r, ih)][32 * c:32 * c + 32, :],
                        s["v"][:, jt, h, :], E[:, c, :],
                        start=(jt == 0), stop=(jt == NT - 1),
                        tile_position=(0, 32 * c))

            emit_scores(0)
            for k in range(nchunks):
                if k + 1 < nchunks:
                    emit_scores(k + 1)
                emit_exp_attnv(k)
                r, ih, jt = chunks[k]
                if jt == NT - 1:                  # last chunk of (r, ih)
                    emit_normalize(b, r, ih, oT.pop((r, ih)),
                                   last=(b == 1 and r == 1 and ih == 1))
                fill(k)
            while fi[0] < len(fillers):           # drain any leftovers
                f = fillers[fi[0]]
                fi[0] += 1
                if f is not None:
                    f()

        # ---------------- schedule ----------------
        # preload the (single) act table while the first DMAs run
        dummy = stat.tile([P, 1], F32, tag="dummy")
        nc.scalar.activation(out=dummy, in_=eps_sb, func=AF.Exp)

        # ---- batch 0 ramp: x loads + LN/tp for it 0-3 + nt0 qk + v0-3 --
        for it in range(4):
            emit_x_load(0, it, q=it % 3)
        for it in range(4, NT):
            emit_x_load(0, it, split=True)
        # warm the PE while the x tiles stream in (it idles ~10us
        # otherwise and the LN-transpose + qk matmuls would run at 1.2GHz)
        warm_tile = ps_scores.tile([P, 2, 512], F32, tag="sc", name="warm")
        emit_junk(warm_tile[:, 0, :], 100)
        for it in range(4):
            emit_ln_stats(0, it)
            emit_rstd(0, 0, k=it)
            emit_ln_norm(0, it)
            emit_tp(0, it)
        emit_qk_chunk(0, 1, 0)   # kT half 0 (scalar-queue reloc)
        emit_qk_chunk(0, 0, 0)   # qT half 0 (sync-queue reloc, parallel)
        for jt in range(4):
            emit_v_chunk(0, jt)

        # ---- fillers for attention(0) ----
        # 32 chunks now; pops: 3/chunk for the first 16, then 2 (80 total).
        pops_a0 = [3] * 16 + [2] * 16
        fill_a0 = []
        # batch-0 prep tail: stats first, one DVE rsqrt for the group,
        # then per-tile norm/transpose/v (pops 1-20)
        for it in range(4, NT):
            fill_a0.append(lambda it=it: emit_ln_stats(0, it))
        fill_a0.append(lambda: emit_rstd_dve(0, 1))
        for it in range(4, NT):
            fill_a0.append(lambda it=it: emit_ln_norm(0, it))
            fill_a0.append(lambda it=it: emit_tp(0, it))
        for it in range(4, NT):
            fill_a0.append(lambda it=it: emit_v_chunk(0, it))
        fill_a0 += [lambda: emit_qk_chunk(0, 1, 1),    # 21-22
                    lambda: emit_qk_chunk(0, 0, 1)]
        for it in range(NT):                           # 23-30
            fill_a0.append(lambda it=it: emit_x_load(1, it, q=it % 2))
        for it in range(4):                            # 31-34
            fill_a0.append(lambda it=it: emit_ln_stats(1, it))
        fill_a0 += [None] * 3                          # 35-37
        fill_a0.append(lambda: emit_rstd_dve(1, 0))    # 38
        for it in range(4):                            # 39-46
            fill_a0.append(lambda it=it: emit_ln_norm(1, it))
            fill_a0.append(lambda it=it: emit_tp(1, it))
        for it in range(4, NT):                        # 47-50
            fill_a0.append(lambda it=it: emit_ln_stats(1, it))
        fill_a0.append(lambda: emit_rstd_dve(1, 1))    # 51
        for it in range(4, NT):                        # 52-59
            fill_a0.append(lambda it=it: emit_ln_norm(1, it))
            fill_a0.append(lambda it=it: emit_tp(1, it))
        fill_a0 += [lambda: emit_qk_chunk(1, 1, 0),    # 60-63
                    lambda: emit_qk_chunk(1, 0, 0),
                    lambda: emit_qk_chunk(1, 1, 1),
                    lambda: emit_qk_chunk(1, 0, 1)]
        fill_a0 += [lambda jt=jt: emit_v_chunk(1, jt)  # 64-71
                    for jt in range(NT)]
        # proj(0, it0-3): valid ~ch 36 (after both ih0 normalizes)
        fill_a0 += [lambda it=it, nt=nt: emit_proj(0, it, nt)   # 72-79
                    for it in range(4) for nt in range(2)]

        emit_attention(0, fill_a0, pops_a0)

        # ---- attention 1 fillers (32 chunks, 1 pop each) ----
        fill_a1 = (
            [None] * 2
            + [lambda it=it, nt=nt: emit_proj(0, it, nt)        # pops 3-10
               for it in range(4, NT) for nt in range(2)]
            + [None] * 9
            + [lambda it=it, nt=nt: emit_proj(1, it, nt)        # pops 20-27
               for it in range(4) for nt in range(2)]
        )
        emit_attention(1, fill_a1, [1] * 32)
        for it in range(4, NT):
            for nt in range(2):
                emit_proj(1, it, nt, q=(it * 2 + nt) % 3)

    nc.finalize()
    return nc


def _prep_weights(gamma, beta, w_qkv, w_proj, b_proj):
    gamma = gamma.astype(np.float64)
    beta = beta.astype(np.float64)
    w_qkv = w_qkv.astype(np.float64)
    w_proj = w_proj.astype(np.float64)
    b_proj = b_proj.astype(np.float64)

    wg = w_qkv * gamma[:, None]
    bias = beta @ w_qkv                   # [384]

    # compact q/k: tile t=0 -> q (SCALE folded), t=1 -> k
    wqk = np.zeros((EMB, 2, P), dtype=np.float64)
    wqk[:, 0, :] = wg[:, :INNER] * SCALE
    wqk[:, 1, :] = wg[:, INNER:2 * INNER]
    bqk = np.zeros((P, 2), dtype=np.float64)
    bqk[:, 0] = bias[:INNER] * SCALE
    bqk[:, 1] = bias[INNER:2 * INNER]
    wqk_t = wqk.reshape(NT, P, 2, P).transpose(1, 0, 2, 3)  # [P, NT, 2, P]

    wv = wg[:, 2 * INNER:3 * INNER].reshape(NT, P, P).transpose(1, 0, 2)
    bv = bias[2 * INNER:3 * INNER].reshape(1, P)

    # o^T row mapping: 32c = ones/rowsum row, 32c+1+d = head (4r+c) dim d
    wpj = np.zeros((P, 2, EMB), dtype=np.float64)
    for r in range(2):
        for c in range(4):
            h = 4 * r + c
            wpj[32 * c + 1:32 * c + 1 + HD, r, :] = \
                w_proj[h * HD:(h + 1) * HD, :]
    wpj[0, 0, :] = b_proj

    bf = ml_dtypes.bfloat16
    return {
        "wqk": np.ascontiguousarray(wqk_t).astype(bf),
        "bqk": np.ascontiguousarray(bqk).astype(np.float32),
        "wv": np.ascontiguousarray(wv).astype(bf),
        "bv": np.ascontiguousarray(bv).astype(bf),
        "wproj": np.ascontiguousarray(wpj).astype(bf),
        "ident": np.eye(P, dtype=np.float32).astype(bf),
    }


def kernel(x, gamma, beta, w_qkv, w_proj, b_proj):
    if "nc" not in _CACHE:
        _CACHE["nc"] = _build()
    nc = _CACHE["nc"]

    w = _prep_weights(gamma, beta, w_qkv, w_proj, b_proj)
    x = np.asarray(x, dtype=np.float32)
    in_maps = []
    for i in range(NCORES):
        m = {"xs": np.ascontiguousarray(x[i * NB:(i + 1) * NB])}
        m.update(w)
        in_maps.append(m)

    res = run_bass_kernel_spmd(nc, in_maps, core_ids=list(range(NCORES)))
    out = np.concatenate([res.results[i]["out"] for i in range(NCORES)], axis=0)
    return out.astype(np.float32)
